# revision 1
# baseline (speedup 1.0000x reference)
"""Trainium2 Bass kernel for nn_MultiHeadFast (multi-head attention with
softmax over the QUERY axis).

Math (faithful to the reference):
  qkv = x @ Ws;  per (b,h):  S[q,k] = Q.K^T,  causal mask k<=q,
  P = softmax_over_q(S * T^-0.5),  out = P @ V.

Key layout trick: compute S TRANSPOSED (S^T[k,q], keys on partitions) so the
query-axis softmax is a free-axis reduction, and S^T is exactly the lhsT
operand needed for out^T = V^T @ P.  The normalizer (per key k) is folded
into V's rows before the PV matmul.  exp has no max-subtraction: |S*c| < 1.5.

Sharding: tensor-parallel over heads.  Core c owns heads {2c, 2c+1}; its Ws
column slice is passed from the host; no collectives.  Everything is bf16
with fp32 accumulation (measured ~5e-3 L2 error vs the fp32 reference).
"""

import numpy as np
from contextlib import ExitStack

import concourse.bass as bass
import concourse.mybir as mybir
import concourse.tile as tile
from concourse import bacc
from concourse.bass_utils import run_bass_kernel_spmd
from concourse.masks import make_identity

B, T, E = 2, 2048, 1024
H, D = 16, 64
NCORES = 8
HPC = H // NCORES            # heads per core = 2
FPC = HPC * D                # feature cols per core per Q/K/V = 128
P = 128
NT = B * T                   # 4096 tokens total
EK = E // P                  # 8 contraction blocks for QKV
NSLAB = T // 512             # 4 query slabs per batch
KTILES = T // P              # 16 key tiles per batch
DT = mybir.dt.bfloat16
F32 = mybir.dt.float32
SCALE = float(T) ** -0.5
NEG = -1e30


def build_kernel():
    nc = bacc.Bacc("TRN2", target_bir_lowering=False, debug=False)
    x_dram = nc.dram_tensor("x", (NT, E), F32, kind="ExternalInput")
    w_dram = nc.dram_tensor("wsl", (E, 3 * FPC), F32, kind="ExternalInput")
    out_dram = nc.dram_tensor("out", (B, T, FPC), F32, kind="ExternalOutput")

    with tile.TileContext(nc) as tc, ExitStack() as ctx:
        const = ctx.enter_context(tc.tile_pool(name="const", bufs=1))
        xtp = ctx.enter_context(tc.tile_pool(name="xtp", bufs=1))
        qkvp = ctx.enter_context(tc.tile_pool(name="qkvp", bufs=1))
        work = ctx.enter_context(tc.tile_pool(name="work", bufs=2))
        strips = ctx.enter_context(tc.tile_pool(name="strips", bufs=4))
        small = ctx.enter_context(tc.tile_pool(name="small", bufs=8))
        outp = ctx.enter_context(tc.tile_pool(name="outp", bufs=3))
        ps = ctx.enter_context(tc.tile_pool(name="ps", bufs=2, space="PSUM"))
        dram = ctx.enter_context(tc.tile_pool(name="dram", bufs=1, space="DRAM"))

        # ---- constants ----
        id_bf = const.tile([P, P], DT, name="id_bf")
        make_identity(nc, id_bf)
        id_f32 = const.tile([P, P], F32, name="id_f32")
        make_identity(nc, id_f32)
        zeros_bf = const.tile([P, P], DT, name="zeros_bf")
        nc.gpsimd.memset(zeros_bf[:], 0.0)
        # diagmask[p, f] = 0 if f >= p else NEG   (keys on partitions, q free)
        diagmask = const.tile([P, P], F32, name="diagmask")
        nc.gpsimd.memset(diagmask[:], 0.0)
        nc.gpsimd.affine_select(
            out=diagmask[:],
            in_=diagmask[:],
            compare_op=mybir.AluOpType.is_ge,
            fill=NEG,
            base=0,
            pattern=[[1, P]],
            channel_multiplier=-1,
        )

        # ---- phase A: x^T (bf16) via cast-DMA + DMA transpose ----
        # phase A strategy: load x fp32 natural (the only input DMA, 16MB),
        # cast to bf16 on GpSimd, transpose 128x128 blocks on the PE, and
        # interleave the QKV matmuls per 512-token slab as x^T becomes ready.
        wsl_f32 = qkvp.tile([P, EK, 3 * FPC], F32, name="wsl_f32")
        nc.sync.dma_start(wsl_f32[:], w_dram.rearrange("(eo ei) f -> ei eo f", ei=P))
        wsl = qkvp.tile([P, EK, 3 * FPC], DT, name="wsl")
        nc.vector.tensor_copy(wsl[:], wsl_f32[:])

        xT = xtp.tile([P, EK, NT], DT, name="xT")
        qt = qkvp.tile([P, NT], DT, name="qt")
        kt_sb = qkvp.tile([P, NT], DT, name="kt_sb")
        vt = qkvp.tile([P, NT], DT, name="vt")
        dsts = [qt, kt_sb, vt]
        for n in range(NT // 512):  # 512-token slabs
            xb = work.tile([P, 4, E], DT, tag="xb", bufs=2, name="xb")
            # SWDGE cast-DMA: fp32 DRAM -> bf16 SBUF, tokens on partitions
            nc.gpsimd.dma_start(
                out=xb[:],
                in_=x_dram[512 * n : 512 * (n + 1), :].rearrange(
                    "(w p) e -> p w e", p=P
                ),
            )
            for w in range(4):
                tp = ps.tile([P, E], DT, tag="pv", bufs=4, name="xtp")
                for e in range(EK):
                    nc.tensor.transpose(
                        tp[:, e * P : (e + 1) * P], xb[:, w, e * P : (e + 1) * P], id_bf[:]
                    )
                cp = nc.scalar.copy if w % 2 == 0 else nc.vector.tensor_copy
                cp(
                    xT[:, :, n * 512 + w * P : n * 512 + (w + 1) * P],
                    tp.rearrange("p (e c) -> p e c", c=P),
                )
            for m in range(3):
                mm_ps = ps.tile([P, 512], F32, tag="pv", bufs=4, name="qkv_ps")
                for e in range(EK):
                    nc.tensor.matmul(
                        mm_ps[:],
                        lhsT=wsl[:, e, m * P : (m + 1) * P],
                        rhs=xT[:, e, n * 512 : (n + 1) * 512],
                        start=(e == 0),
                        stop=(e == EK - 1),
                    )
                nc.scalar.copy(dsts[m][:, n * 512 : (n + 1) * 512], mm_ps[:])

        # ---- phase C: V^T -> V (tokens on partitions), per (b, hh) ----
        v_nat = qkvp.tile([P, B * HPC, KTILES, D], DT, name="v_nat")
        for b in range(B):
            for hh in range(HPC):
                for k in range(KTILES):
                    tok0 = b * T + k * P
                    tps = ps.tile([P, D], DT, tag="pv", bufs=4, name="vtp")
                    nc.tensor.transpose(
                        tps[:],
                        vt[hh * D : (hh + 1) * D, tok0 : tok0 + P],
                        id_bf[hh * D : (hh + 1) * D, hh * D : (hh + 1) * D],
                    )
                    nc.vector.tensor_copy(v_nat[:, b * HPC + hh, k, :], tps[:])

        # ---- phase D: attention per batch (software-pipelined over k) ----
        for b in range(B):
            pv_ps = [
                ps.tile([P, 512], F32, tag="pv", bufs=4, name=f"pv_{b}_{j}")
                for j in range(NSLAB)
            ]
            # Zero-initialize each PV accumulator bank with a full-width
            # zero matmul so every partition row's has_written state is set
            # identically under both the per-row and whole-bank semantics;
            # all real PV matmuls then accumulate with start=False.
            for j in range(NSLAB):
                nc.tensor.matmul(
                    pv_ps[j][:],
                    lhsT=zeros_bf[:],
                    rhs=qt[:, b * T : b * T + 512],
                    start=True,
                    stop=False,
                    skip_group_check=True,
                )

            def chunk_mms(b, k, hh, strip, coff, cw):
                """S^T matmuls + mask + exp for one chunk of a head strip."""
                j0 = k // 4
                q0 = 512 * j0
                dead = P * k - q0
                sps = ps.tile([P, 1024], F32, tag="sps", bufs=2, name="sps")
                for so in range(0, cw, 512):
                    qs = q0 + coff + so
                    nc.tensor.matmul(
                        sps[:, so : so + 512],
                        lhsT=kt_sb[hh * D : (hh + 1) * D, b * T + k * P : b * T + k * P + P],
                        rhs=qt[hh * D : (hh + 1) * D, b * T + qs : b * T + qs + 512],
                        start=True,
                        stop=True,
                    )
                acc = small.tile([P, 1], F32, tag="acc", name="acc")
                if coff == 0:
                    nc.vector.tensor_add(
                        sps[:, dead : dead + P], sps[:, dead : dead + P], diagmask[:]
                    )
                    if dead > 0:
                        nc.gpsimd.memset(strip[:, 0:dead], 0.0)
                    nc.scalar.activation(
                        strip[:, dead:cw],
                        sps[:, dead:cw],
                        mybir.ActivationFunctionType.Exp,
                        scale=SCALE,
                        accum_out=acc[:],
                    )
                else:
                    nc.scalar.activation(
                        strip[:, coff : coff + cw],
                        sps[:, :cw],
                        mybir.ActivationFunctionType.Exp,
                        scale=SCALE,
                        accum_out=acc[:],
                    )
                return acc

            def finish_head(b, k, hh, partials):
                if len(partials) == 1:
                    ssum = partials[0]
                else:
                    ssum = small.tile([P, 1], F32, tag="acc", name="ssum")
                    nc.vector.tensor_add(ssum[:], partials[0][:], partials[1][:])
                rsum = small.tile([P, 1], F32, tag="acc", name="rsum")
                nc.vector.reciprocal(rsum[:], ssum[:])
                vp = small.tile([P, D], DT, tag="vp", name="vp")
                nc.vector.tensor_scalar_mul(
                    vp[:], v_nat[:, b * HPC + hh, k, :], rsum[:]
                )
                return vp

            def pv_head(b, k, hh, strip, vp):
                j0 = k // 4
                q0 = 512 * j0
                for j in range(j0, NSLAB):
                    nc.tensor.matmul(
                        pv_ps[j][hh * D : (hh + 1) * D, :],
                        lhsT=vp[:],
                        rhs=strip[:, 512 * j - q0 : 512 * j - q0 + 512],
                        start=False,
                        stop=(k == 4 * j + 3 and hh == HPC - 1),
                        skip_group_check=True,
                    )

            # software pipeline: chunk-level head alternation keeps 2 chunks
            # in flight (one per head) so the ACT exp stream never starves;
            # PV matmuls of k-1 fill the PE between chunk groups.
            prev = {}
            for k in range(KTILES):
                j0 = k // 4
                L = T - 512 * j0
                strip_k = {}
                parts = {0: [], 1: []}
                for hh in range(HPC):
                    strip_k[hh] = strips.tile([P, T], DT, tag="strip", name=f"s{hh}")
                coff = 0
                while coff < L:
                    cw = min(1024, L - coff)
                    for hh in range(HPC):
                        parts[hh].append(chunk_mms(b, k, hh, strip_k[hh], coff, cw))
                    coff += cw
                for hh in range(HPC):
                    vp = finish_head(b, k, hh, parts[hh])
                    if k > 0:
                        pv_head(b, k - 1, hh, *prev[hh])
                    prev[hh] = (strip_k[hh], vp)
            for hh in range(HPC):
                pv_head(b, KTILES - 1, hh, *prev[hh])
            # evacuate + transpose out^T -> out
            for j in range(NSLAB):
                osb = outp.tile([P, 512], F32, tag="osb", name="osb")
                nc.vector.tensor_copy(osb[:], pv_ps[j][:])
                o_sb = outp.tile([P, 4, P], F32, tag="o_sb", name="o_sb")
                for w in range(4):
                    tp = ps.tile([P, P], F32, tag="pv", bufs=4, name="otp")
                    nc.tensor.transpose(tp[:], osb[:, w * P : (w + 1) * P], id_f32[:])
                    nc.vector.tensor_copy(o_sb[:, w, :], tp[:])
                nc.sync.dma_start(
                    out_dram[b, 512 * j : 512 * (j + 1), :].rearrange(
                        "(w p) f -> p w f", p=P
                    ),
                    o_sb[:],
                )
    nc.compile()
    return nc


_NC_CACHE = None


def kernel(x: np.ndarray, Ws: np.ndarray) -> np.ndarray:
    global _NC_CACHE
    if _NC_CACHE is None:
        _NC_CACHE = build_kernel()
    nc = _NC_CACHE

    x2 = np.ascontiguousarray(x.reshape(NT, E).astype(np.float32, copy=False))
    in_maps = []
    for c in range(NCORES):
        cols = np.concatenate(
            [
                Ws[:, c * FPC : (c + 1) * FPC],
                Ws[:, E + c * FPC : E + (c + 1) * FPC],
                Ws[:, 2 * E + c * FPC : 2 * E + (c + 1) * FPC],
            ],
            axis=1,
        ).astype(np.float32, copy=False)
        in_maps.append({"x": x2, "wsl": np.ascontiguousarray(cols)})

    res = run_bass_kernel_spmd(nc, in_maps, core_ids=list(range(NCORES)))
    out = np.empty((B, T, H * D), np.float32)
    for c in range(NCORES):
        out[:, :, c * FPC : (c + 1) * FPC] = res.results[c]["out"]
    return out



# revision 4
# speedup vs baseline: 1.0906x; 1.0906x over previous
"""Trainium2 Bass kernel for nn_MultiHeadFast (multi-head attention with
softmax over the QUERY axis).

Math (faithful to the reference):
  qkv = x @ Ws;  per (b,h):  S[q,k] = Q.K^T,  causal mask k<=q,
  P = softmax_over_q(S * T^-0.5),  out = P @ V.

Layout strategy (v2):
  * Host passes x TRANSPOSED and in bf16: xT (E, NT).  The QKV matmuls need
    E on partitions for both operands, so transposing on the host removes
    256 PE transposes + their PSUM evacuations from the device entirely.
  * Q^T / K^T are computed feature-on-partition (2 heads * 64 = 128 rows);
    V is computed directly in NATURAL layout (tokens on partitions) since
    the PV matmul wants V as its stationary [keys, d] operand.
  * S is computed TRANSPOSED (S^T[k, q], keys on partitions) so the
    query-axis softmax is a free-axis reduction (ACT accum during exp).
  * Strips start exactly at the 128-aligned causal diagonal; the diagonal
    128x128 block is masked by adding -1e30 (DVE); exp of that is 0.
  * PV accumulates out^T = (V/sigma)^T @ P^T into 4 PSUM slab banks; the
    diagonal slab matmul is ragged (skips the dead columns), so no strip
    memsets are needed.  out^T is DMA'd out and transposed on the host.
  * exp has no max-subtraction: |S*c| < 1.5.  Everything is bf16 with fp32
    accumulation.

Sharding: tensor-parallel over heads.  Core c owns heads {2c, 2c+1}; no
collectives.
"""

import numpy as np
import ml_dtypes
from contextlib import ExitStack

import concourse.bass as bass
import concourse.mybir as mybir
import concourse.tile as tile
from concourse import bacc
from concourse.bass_utils import run_bass_kernel_spmd
from concourse.masks import make_identity

B, T, E = 2, 2048, 1024
H, D = 16, 64
NCORES = 8
HPC = H // NCORES            # heads per core = 2
FPC = HPC * D                # feature cols per core per Q/K/V = 128
P = 128
NT = B * T                   # 4096 tokens total
EK = E // P                  # 8 contraction blocks for QKV
KTILES = T // P              # 16 key tiles per batch
NSLAB = T // 512             # 4 query slabs per batch
DT = mybir.dt.bfloat16
F32 = mybir.dt.float32
SCALE = float(T) ** -0.5
NEG = -1e30


def build_kernel():
    nc = bacc.Bacc("TRN2", target_bir_lowering=False, debug=False)
    xt_dram = nc.dram_tensor("xt", (E, NT), DT, kind="ExternalInput")
    w_dram = nc.dram_tensor("wsl", (E, 3 * FPC), DT, kind="ExternalInput")
    out_dram = nc.dram_tensor("out", (FPC, NT), F32, kind="ExternalOutput")

    with tile.TileContext(nc) as tc, ExitStack() as ctx:
        const = ctx.enter_context(tc.tile_pool(name="const", bufs=1))
        big = ctx.enter_context(tc.tile_pool(name="big", bufs=1))
        strips = ctx.enter_context(tc.tile_pool(name="strips", bufs=4))
        small = ctx.enter_context(tc.tile_pool(name="small", bufs=8))
        outp = ctx.enter_context(tc.tile_pool(name="outp", bufs=2))
        ps = ctx.enter_context(tc.tile_pool(name="ps", bufs=2, space="PSUM"))

        # ---- constants ----
        zeros_bf = const.tile([P, P], DT, name="zeros_bf")
        nc.gpsimd.memset(zeros_bf[:], 0.0)
        # diagmask[p, f] = 0 if f >= p else NEG   (keys on partitions, q free)
        diagmask = const.tile([P, P], F32, name="diagmask")
        nc.gpsimd.memset(diagmask[:], 0.0)
        nc.gpsimd.affine_select(
            out=diagmask[:],
            in_=diagmask[:],
            compare_op=mybir.AluOpType.is_ge,
            fill=NEG,
            base=0,
            pattern=[[1, P]],
            channel_multiplier=-1,
        )

        # ---- persistent SBUF tensors ----
        wsl = big.tile([P, EK, 3 * FPC], DT, name="wsl")
        nc.sync.dma_start(wsl[:], w_dram.rearrange("(eo ei) f -> ei eo f", ei=P))
        xT = big.tile([P, EK, NT], DT, name="xT")
        qkt = big.tile([P, 2, NT], DT, name="qkt")   # [:,0,:]=Q^T  [:,1,:]=K^T
        v_nat = big.tile([P, B * KTILES, FPC], DT, name="v_nat")
        xt_view = xt_dram.rearrange("(eo ei) t -> ei eo t", ei=P)

        def phase_a_slab(n):
            """QKV for one 512-token slab: Q^T,K^T + V natural."""
            t0 = 512 * n
            nc.sync.dma_start(xT[:, :, t0 : t0 + 512], xt_view[:, :, t0 : t0 + 512])
            qk_ps = ps.tile([P, 1024], F32, tag="sps", bufs=2, name="qk_ps")
            for m in range(2):
                for e in range(EK):
                    nc.tensor.matmul(
                        qk_ps[:, m * 512 : (m + 1) * 512],
                        lhsT=wsl[:, e, m * P : (m + 1) * P],
                        rhs=xT[:, e, t0 : t0 + 512],
                        start=(e == 0),
                        stop=(e == EK - 1),
                    )
            nc.vector.tensor_copy(
                qkt[:, :, t0 : t0 + 512],
                qk_ps.rearrange("p (m t) -> p m t", m=2),
            )
            v_ps = ps.tile([P, 1024], F32, tag="sps", bufs=2, name="v_ps")
            for tb in range(4):
                for e in range(EK):
                    nc.tensor.matmul(
                        v_ps[:, tb * P : (tb + 1) * P],
                        lhsT=xT[:, e, t0 + tb * P : t0 + (tb + 1) * P],
                        rhs=wsl[:, e, 2 * FPC : 3 * FPC],
                        start=(e == 0),
                        stop=(e == EK - 1),
                    )
            nc.vector.tensor_copy(
                v_nat[:, 4 * n : 4 * n + 4, :],
                v_ps[:, 0:512].rearrange("p (tb f) -> p tb f", tb=4),
            )

        def s_chunk(b, k, hh, strip, coff, cw):
            """S^T matmuls + mask + exp for one <=1024-wide chunk of a strip."""
            q0 = P * k
            sps = ps.tile([P, 1024], F32, tag="sps", bufs=2, name="sps")
            for so in range(0, cw, 512):
                w = min(512, cw - so)
                qs = b * T + q0 + coff + so
                nc.tensor.matmul(
                    sps[:, so : so + w],
                    lhsT=qkt[hh * D : (hh + 1) * D, 1, b * T + q0 : b * T + q0 + P],
                    rhs=qkt[hh * D : (hh + 1) * D, 0, qs : qs + w],
                    start=True,
                    stop=True,
                )
            if coff == 0:
                nc.vector.tensor_add(sps[:, 0:P], sps[:, 0:P], diagmask[:])
            acc = small.tile([P, 1], F32, tag="acc", name="acc")
            nc.scalar.activation(
                strip[:, coff : coff + cw],
                sps[:, :cw],
                mybir.ActivationFunctionType.Exp,
                scale=SCALE,
                accum_out=acc[:],
            )
            return acc

        def finish_head(b, k, hh, partials):
            if len(partials) == 1:
                ssum = partials[0]
            else:
                ssum = small.tile([P, 1], F32, tag="acc", name="ssum")
                nc.vector.tensor_add(ssum[:], partials[0][:], partials[1][:])
            rsum = small.tile([P, 1], F32, tag="acc", name="rsum")
            nc.vector.reciprocal(rsum[:], ssum[:])
            vp = small.tile([P, D], DT, tag="vp", name="vp")
            nc.vector.tensor_scalar_mul(
                vp[:], v_nat[:, b * KTILES + k, hh * D : (hh + 1) * D], rsum[:]
            )
            return vp

        def pv_head(b, k, hh, strip, vp, pv_ps):
            q0 = P * k
            j0 = k // 4
            dead = q0 - 512 * j0
            for j in range(j0, NSLAB):
                stop = k == 4 * j + 3 and hh == HPC - 1
                if j == j0:
                    nc.tensor.matmul(
                        pv_ps[j][hh * D : (hh + 1) * D, dead:512],
                        lhsT=vp[:],
                        rhs=strip[:, 0 : 512 - dead],
                        start=False,
                        stop=stop,
                        skip_group_check=True,
                    )
                else:
                    nc.tensor.matmul(
                        pv_ps[j][hh * D : (hh + 1) * D, :],
                        lhsT=vp[:],
                        rhs=strip[:, 512 * j - q0 : 512 * j - q0 + 512],
                        start=False,
                        stop=stop,
                        skip_group_check=True,
                    )

        def attn_batch(b, a_slabs):
            """Attention for batch b; a_slabs maps k -> list of phase-A slab
            indices (for the other batch) to interleave at that point."""
            pv_ps = [
                ps.tile([P, 512], F32, tag="pv", bufs=4, name=f"pv_{b}_{j}")
                for j in range(NSLAB)
            ]
            for j in range(NSLAB):
                nc.tensor.matmul(
                    pv_ps[j][:],
                    lhsT=zeros_bf[:],
                    rhs=qkt[:, 0, b * T : b * T + 512],
                    start=True,
                    stop=False,
                    skip_group_check=True,
                )
            prev = {}
            for k in range(KTILES):
                for n in a_slabs.get(k, ()):
                    phase_a_slab(n)
                L = T - P * k
                strip_k = {}
                parts = {0: [], 1: []}
                for hh in range(HPC):
                    strip_k[hh] = strips.tile([P, T], DT, tag="strip", name=f"s{hh}")
                coff = 0
                while coff < L:
                    cw = min(1024, L - coff)
                    for hh in range(HPC):
                        parts[hh].append(
                            s_chunk(b, k, hh, strip_k[hh], coff, cw)
                        )
                    coff += cw
                for hh in range(HPC):
                    vp = finish_head(b, k, hh, parts[hh])
                    if k > 0:
                        pv_head(b, k - 1, hh, *prev[hh], pv_ps)
                    prev[hh] = (strip_k[hh], vp)
            for hh in range(HPC):
                pv_head(b, KTILES - 1, hh, *prev[hh], pv_ps)
            for j in range(NSLAB):
                osb = outp.tile([P, 512], F32, tag="osb", name="osb")
                nc.vector.tensor_copy(osb[:], pv_ps[j][:])
                nc.sync.dma_start(
                    out_dram[:, b * T + 512 * j : b * T + 512 * (j + 1)], osb[:]
                )

        # program order: batch-0 QKV, then batch-0 attention with batch-1
        # QKV slabs interleaved into its first k iterations, then batch-1
        # attention.
        for n in range(4):
            phase_a_slab(n)
        attn_batch(0, {0: [4], 2: [5], 4: [6], 6: [7]})
        attn_batch(1, {})
    nc.compile()
    return nc


_NC_CACHE = None


def make_in_maps(x: np.ndarray, Ws: np.ndarray) -> list:
    xt = np.ascontiguousarray(x.reshape(NT, E).T.astype(ml_dtypes.bfloat16))
    in_maps = []
    for c in range(NCORES):
        cols = np.concatenate(
            [
                Ws[:, c * FPC : (c + 1) * FPC],
                Ws[:, E + c * FPC : E + (c + 1) * FPC],
                Ws[:, 2 * E + c * FPC : 2 * E + (c + 1) * FPC],
            ],
            axis=1,
        ).astype(ml_dtypes.bfloat16)
        in_maps.append({"xt": xt, "wsl": np.ascontiguousarray(cols)})
    return in_maps


def assemble_out(results: list) -> np.ndarray:
    out = np.empty((B, T, H * D), np.float32)
    for c in range(NCORES):
        r = results[c]["out"].reshape(FPC, B, T)
        for b in range(B):
            out[b, :, c * FPC : (c + 1) * FPC] = r[:, b, :].T
    return out


def kernel(x: np.ndarray, Ws: np.ndarray) -> np.ndarray:
    global _NC_CACHE
    if _NC_CACHE is None:
        _NC_CACHE = build_kernel()
    nc = _NC_CACHE
    res = run_bass_kernel_spmd(
        nc, make_in_maps(x, Ws), core_ids=list(range(NCORES))
    )
    return assemble_out(res.results)


# revision 11
# speedup vs baseline: 1.1476x; 1.0522x over previous
"""Trainium2 Bass kernel for nn_MultiHeadFast (multi-head attention with
softmax over the QUERY axis).

Math (faithful to the reference):
  qkv = x @ Ws;  per (b,h):  S[q,k] = Q.K^T,  causal mask k<=q,
  P = softmax_over_q(S * T^-0.5),  out = P @ V.

Layout strategy (v2):
  * Host passes x TRANSPOSED and in bf16: xT (E, NT).  The QKV matmuls need
    E on partitions for both operands, so transposing on the host removes
    256 PE transposes + their PSUM evacuations from the device entirely.
  * Q^T / K^T are computed feature-on-partition (2 heads * 64 = 128 rows);
    V is computed directly in NATURAL layout (tokens on partitions) since
    the PV matmul wants V as its stationary [keys, d] operand.
  * S is computed TRANSPOSED (S^T[k, q], keys on partitions) so the
    query-axis softmax is a free-axis reduction (ACT accum during exp).
  * Strips start exactly at the 128-aligned causal diagonal; the diagonal
    128x128 block is masked by adding -1e30 (DVE); exp of that is 0.
  * PV accumulates out^T = (V/sigma)^T @ P^T into 4 PSUM slab banks; the
    diagonal slab matmul is ragged (skips the dead columns), so no strip
    memsets are needed.  out^T is DMA'd out and transposed on the host.
  * exp has no max-subtraction: |S*c| < 1.5.  Everything is bf16 with fp32
    accumulation.

Sharding: tensor-parallel over heads.  Core c owns heads {2c, 2c+1}; no
collectives.
"""

import numpy as np
import ml_dtypes
from contextlib import ExitStack

import concourse.bass as bass
import concourse.mybir as mybir
import concourse.tile as tile
from concourse import bacc
from concourse.bass_utils import run_bass_kernel_spmd
from concourse.masks import make_identity

B, T, E = 2, 2048, 1024
H, D = 16, 64
NCORES = 8
HPC = H // NCORES            # heads per core = 2
FPC = HPC * D                # feature cols per core per Q/K/V = 128
P = 128
NT = B * T                   # 4096 tokens total
EK = E // P                  # 8 contraction blocks for QKV
KTILES = T // P              # 16 key tiles per batch
NSLAB = T // 512             # 4 query slabs per batch
DT = mybir.dt.bfloat16
F32 = mybir.dt.float32
SCALE = float(T) ** -0.5
NEG = -1e30


def build_kernel():
    nc = bacc.Bacc("TRN2", target_bir_lowering=False, debug=False)
    xt_dram = nc.dram_tensor("xt", (E, NT), DT, kind="ExternalInput")
    w_dram = nc.dram_tensor("wsl", (E, 3 * FPC), DT, kind="ExternalInput")
    out_dram = nc.dram_tensor("out", (FPC, NT), F32, kind="ExternalOutput")

    with tile.TileContext(nc) as tc, ExitStack() as ctx:
        const = ctx.enter_context(tc.tile_pool(name="const", bufs=1))
        big = ctx.enter_context(tc.tile_pool(name="big", bufs=1))
        strips = ctx.enter_context(tc.tile_pool(name="strips", bufs=4))
        small = ctx.enter_context(tc.tile_pool(name="small", bufs=8))
        outp = ctx.enter_context(tc.tile_pool(name="outp", bufs=2))
        ps = ctx.enter_context(tc.tile_pool(name="ps", bufs=2, space="PSUM"))

        # ---- constants ----
        zeros_bf = const.tile([P, P], DT, name="zeros_bf")
        nc.gpsimd.memset(zeros_bf[:], 0.0)
        id_bf = const.tile([P, P], DT, name="id_bf")
        make_identity(nc, id_bf)
        # tri_neg[c, m] = NEG if c < m else 0.  Used as matmul stationary to
        # seed the causal mask into the S PSUM group on the PE itself:
        # (tri_neg^T @ I)[p, f] = tri_neg[f, p] = NEG iff f < p.
        tri_neg = const.tile([P, P], DT, name="tri_neg")
        nc.gpsimd.memset(tri_neg[:], 0.0)
        nc.gpsimd.affine_select(
            out=tri_neg[:],
            in_=tri_neg[:],
            compare_op=mybir.AluOpType.is_ge,
            fill=NEG,
            base=0,
            pattern=[[-1, P]],
            channel_multiplier=1,
        )

        # ---- persistent SBUF tensors ----
        wsl = big.tile([P, EK, 3 * FPC], DT, name="wsl")
        nc.sync.dma_start(wsl[:], w_dram.rearrange("(eo ei) f -> ei eo f", ei=P))
        xT = big.tile([P, EK, NT], DT, name="xT")
        qkt = big.tile([P, 2, NT], DT, name="qkt")   # [:,0,:]=Q^T  [:,1,:]=K^T
        v_nat = big.tile([P, B * KTILES, FPC], DT, name="v_nat")
        xt_view = xt_dram.rearrange("(eo ei) t -> ei eo t", ei=P)

        def phase_a_slab(n):
            """QKV for one 512-token slab: Q^T,K^T + V natural."""
            t0 = 512 * n
            nc.sync.dma_start(xT[:, :, t0 : t0 + 512], xt_view[:, :, t0 : t0 + 512])
            qk_ps = ps.tile([P, 1024], F32, tag="sps", bufs=2, name="qk_ps")
            for m in range(2):
                for e in range(EK):
                    nc.tensor.matmul(
                        qk_ps[:, m * 512 : (m + 1) * 512],
                        lhsT=wsl[:, e, m * P : (m + 1) * P],
                        rhs=xT[:, e, t0 : t0 + 512],
                        start=(e == 0),
                        stop=(e == EK - 1),
                    )
            nc.vector.tensor_copy(
                qkt[:, :, t0 : t0 + 512],
                qk_ps.rearrange("p (m t) -> p m t", m=2),
            )
            v_ps = ps.tile([P, 1024], F32, tag="sps", bufs=2, name="v_ps")
            for tb in range(4):
                for e in range(EK):
                    nc.tensor.matmul(
                        v_ps[:, tb * P : (tb + 1) * P],
                        lhsT=xT[:, e, t0 + tb * P : t0 + (tb + 1) * P],
                        rhs=wsl[:, e, 2 * FPC : 3 * FPC],
                        start=(e == 0),
                        stop=(e == EK - 1),
                    )
            nc.vector.tensor_copy(
                v_nat[:, 4 * n : 4 * n + 4, :],
                v_ps[:, 0:512].rearrange("p (tb f) -> p tb f", tb=4),
            )

        def s_chunk(b, k, hh, strip, coff, cw):
            """S^T matmuls (+ causal-mask PSUM seed) + exp for one <=1024-wide
            chunk of a strip."""
            q0 = P * k
            kt = qkt[hh * D : (hh + 1) * D, 1, b * T + q0 : b * T + q0 + P]
            sps = ps.tile([P, 1024], F32, tag="sps", bufs=2, name="sps")
            s = coff
            if coff == 0:
                # diagonal block: seed the mask on the PE, then accumulate S
                nc.tensor.matmul(
                    sps[:, 0:P], lhsT=tri_neg[:], rhs=id_bf[:],
                    start=True, stop=False,
                )
                nc.tensor.matmul(
                    sps[:, 0:P],
                    lhsT=kt,
                    rhs=qkt[hh * D : (hh + 1) * D, 0, b * T + q0 : b * T + q0 + P],
                    start=False,
                    stop=True,
                )
                s = P
            while s < coff + cw:
                e = min(coff + cw, (s // 512 + 1) * 512)
                nc.tensor.matmul(
                    sps[:, s - coff : e - coff],
                    lhsT=kt,
                    rhs=qkt[hh * D : (hh + 1) * D, 0, b * T + q0 + s : b * T + q0 + e],
                    start=True,
                    stop=True,
                )
                s = e
            acc = small.tile([P, 1], F32, tag="acc", name="acc")
            nc.scalar.activation(
                strip[:, coff : coff + cw],
                sps[:, :cw],
                mybir.ActivationFunctionType.Exp,
                scale=SCALE,
                accum_out=acc[:],
            )
            return acc

        def finish_head(b, k, hh, partials):
            if len(partials) == 1:
                ssum = partials[0]
            else:
                ssum = small.tile([P, 1], F32, tag="acc", name="ssum")
                nc.vector.tensor_add(ssum[:], partials[0][:], partials[1][:])
            rsum = small.tile([P, 1], F32, tag="acc", name="rsum")
            nc.vector.reciprocal(rsum[:], ssum[:])
            vp = small.tile([P, D], DT, tag="vp", name="vp")
            nc.vector.tensor_scalar_mul(
                vp[:], v_nat[:, b * KTILES + k, hh * D : (hh + 1) * D], rsum[:]
            )
            return vp

        def pv_head(b, k, hh, strip, vp, pv_ps):
            q0 = P * k
            j0 = k // 4
            dead = q0 - 512 * j0
            for j in range(j0, NSLAB):
                # descending k-loop: the last contributor to every slab is k=0
                stop = k == 0 and hh == HPC - 1
                if j == j0:
                    nc.tensor.matmul(
                        pv_ps[j][hh * D : (hh + 1) * D, dead:512],
                        lhsT=vp[:],
                        rhs=strip[:, 0 : 512 - dead],
                        start=False,
                        stop=stop,
                        skip_group_check=True,
                    )
                else:
                    nc.tensor.matmul(
                        pv_ps[j][hh * D : (hh + 1) * D, :],
                        lhsT=vp[:],
                        rhs=strip[:, 512 * j - q0 : 512 * j - q0 + 512],
                        start=False,
                        stop=stop,
                        skip_group_check=True,
                    )

        def attn_batch(b, a_slabs):
            """Attention for batch b; a_slabs maps k -> list of phase-A slab
            indices (for the other batch) to interleave at that point."""
            pv_ps = [
                ps.tile([P, 512], F32, tag="pv", bufs=4, name=f"pv_{b}_{j}")
                for j in range(NSLAB)
            ]
            # rhs must be initialized data (0 * NaN = NaN): wsl is loaded first
            for j in range(NSLAB):
                nc.tensor.matmul(
                    pv_ps[j][:],
                    lhsT=zeros_bf[:],
                    rhs=wsl.rearrange("p e f -> p (e f)")[:, 0:512],
                    start=True,
                    stop=False,
                    skip_group_check=True,
                )
            prev = {}
            for k in range(KTILES - 1, -1, -1):
                for n in a_slabs.get(k, ()):
                    phase_a_slab(n)
                L = T - P * k
                strip_k = {}
                parts = {0: [], 1: []}
                for hh in range(HPC):
                    strip_k[hh] = strips.tile([P, T], DT, tag="strip", name=f"s{hh}")
                coff = 0
                while coff < L:
                    cw = min(1024, L - coff)
                    for hh in range(HPC):
                        parts[hh].append(
                            s_chunk(b, k, hh, strip_k[hh], coff, cw)
                        )
                    coff += cw
                for hh in range(HPC):
                    vp = finish_head(b, k, hh, parts[hh])
                    if k < KTILES - 1:
                        pv_head(b, k + 1, hh, *prev[hh], pv_ps)
                    prev[hh] = (strip_k[hh], vp)
            for hh in range(HPC):
                pv_head(b, 0, hh, *prev[hh], pv_ps)
            for j in range(NSLAB):
                osb = outp.tile([P, 512], F32, tag="osb", name="osb")
                nc.vector.tensor_copy(osb[:], pv_ps[j][:])
                nc.sync.dma_start(
                    out_dram[:, b * T + 512 * j : b * T + 512 * (j + 1)], osb[:]
                )

        # Program order: the k-loop runs DESCENDING, so attention for batch 0
        # can start as soon as its last QKV slab (3) is done; the remaining
        # slabs stream in down the k-loop just before each is first needed
        # (k>=12 needs slab 3 only; k>=8 slabs 2-3; k>=4 slabs 1-3), and
        # batch-1 slabs fill the PE during the ACT-bound middle.
        phase_a_slab(3)
        attn_batch(
            0, {14: [2], 12: [1], 10: [0], 8: [7], 6: [6], 4: [5], 2: [4]}
        )
        attn_batch(1, {})
    nc.compile()
    return nc


_NC_CACHE = None


def make_in_maps(x: np.ndarray, Ws: np.ndarray) -> list:
    xt = np.ascontiguousarray(x.reshape(NT, E).T.astype(ml_dtypes.bfloat16))
    in_maps = []
    for c in range(NCORES):
        cols = np.concatenate(
            [
                Ws[:, c * FPC : (c + 1) * FPC],
                Ws[:, E + c * FPC : E + (c + 1) * FPC],
                Ws[:, 2 * E + c * FPC : 2 * E + (c + 1) * FPC],
            ],
            axis=1,
        ).astype(ml_dtypes.bfloat16)
        in_maps.append({"xt": xt, "wsl": np.ascontiguousarray(cols)})
    return in_maps


def assemble_out(results: list) -> np.ndarray:
    out = np.empty((B, T, H * D), np.float32)
    for c in range(NCORES):
        r = results[c]["out"].reshape(FPC, B, T)
        for b in range(B):
            out[b, :, c * FPC : (c + 1) * FPC] = r[:, b, :].T
    return out


def kernel(x: np.ndarray, Ws: np.ndarray) -> np.ndarray:
    global _NC_CACHE
    if _NC_CACHE is None:
        _NC_CACHE = build_kernel()
    nc = _NC_CACHE
    res = run_bass_kernel_spmd(
        nc, make_in_maps(x, Ws), core_ids=list(range(NCORES))
    )
    return assemble_out(res.results)


# revision 13
# speedup vs baseline: 1.2229x; 1.0656x over previous
"""Trainium2 Bass kernel for nn_MultiHeadFast (multi-head attention with
softmax over the QUERY axis).

Math (faithful to the reference):
  qkv = x @ Ws;  per (b,h):  S[q,k] = Q.K^T,  causal mask k<=q,
  P = softmax_over_q(S * T^-0.5),  out = P @ V.

Layout strategy (v4):
  * Host passes x TRANSPOSED and in bf16: xT (E, NT); device never
    transposes x.  Q^T / K^T are computed feature-on-partition; V is
    computed directly in NATURAL layout (tokens on partitions) for the PV
    stationary.  out^T is DMA'd out and transposed on the host.
  * S is computed TRANSPOSED (S^T[k, q], keys on partitions) so the
    query-axis softmax is a free-axis reduction (ACT accum during exp).
    Strips start exactly at the 128-aligned causal diagonal; the causal
    mask of the diagonal block is seeded INTO the S PSUM group by an extra
    matmul (tri_neg^T @ I), keeping the S->exp critical path PE-only.
  * QKV work is cut into small pieces (one PSUM tile each) that are
    interleaved down the attention k-loop via a deadline schedule, so the
    ACT engine never starves behind a monolithic QKV block and the PE
    stays busy (and at full clock) through the ACT-bound phase.
  * attn(b=0) runs its k-loop DESCENDING: k=15 needs only the last 128
    tokens of QKV, so attention starts ~4us in.  attn(b=1) runs ASCENDING:
    its PSUM output banks retire one-by-one (k=4j+3), spreading the output
    DMAs and shrinking the tail.
  * exp has no max-subtraction: |S*c| < 1.5.  bf16 with fp32 accumulation.

Sharding: tensor-parallel over heads.  Core c owns heads {2c, 2c+1}; no
collectives.
"""

import numpy as np
import ml_dtypes
from contextlib import ExitStack

import concourse.bass as bass
import concourse.mybir as mybir
import concourse.tile as tile
from concourse import bacc
from concourse.bass_utils import run_bass_kernel_spmd
from concourse.masks import make_identity

B, T, E = 2, 2048, 1024
H, D = 16, 64
NCORES = 8
HPC = H // NCORES            # heads per core = 2
FPC = HPC * D                # feature cols per core per Q/K/V = 128
P = 128
NT = B * T                   # 4096 tokens total
EK = E // P                  # 8 contraction blocks for QKV
KTILES = T // P              # 16 key tiles per batch
NSLAB = T // 512             # 4 query slabs per batch
DT = mybir.dt.bfloat16
F32 = mybir.dt.float32
SCALE = float(T) ** -0.5
NEG = -1e30


def build_kernel():
    nc = bacc.Bacc("TRN2", target_bir_lowering=False, debug=False)
    xt_dram = nc.dram_tensor("xt", (E, NT), DT, kind="ExternalInput")
    w_dram = nc.dram_tensor("wsl", (E, 3 * FPC), DT, kind="ExternalInput")
    out_dram = nc.dram_tensor("out", (FPC, NT), F32, kind="ExternalOutput")

    with tile.TileContext(nc) as tc, ExitStack() as ctx:
        const = ctx.enter_context(tc.tile_pool(name="const", bufs=1))
        big = ctx.enter_context(tc.tile_pool(name="big", bufs=1))
        strips = ctx.enter_context(tc.tile_pool(name="strips", bufs=4))
        small = ctx.enter_context(tc.tile_pool(name="small", bufs=8))
        outp = ctx.enter_context(tc.tile_pool(name="outp", bufs=2))
        ps = ctx.enter_context(tc.tile_pool(name="ps", bufs=2, space="PSUM"))

        # ---- constants ----
        zeros_bf = const.tile([P, P], DT, name="zeros_bf")
        nc.gpsimd.memset(zeros_bf[:], 0.0)
        id_bf = const.tile([P, P], DT, name="id_bf")
        make_identity(nc, id_bf)
        # tri_neg[c, m] = NEG if c < m else 0.  Used as matmul stationary to
        # seed the causal mask into the S PSUM group on the PE itself:
        # (tri_neg^T @ I)[p, f] = tri_neg[f, p] = NEG iff f < p.
        tri_neg = const.tile([P, P], DT, name="tri_neg")
        nc.gpsimd.memset(tri_neg[:], 0.0)
        nc.gpsimd.affine_select(
            out=tri_neg[:],
            in_=tri_neg[:],
            compare_op=mybir.AluOpType.is_ge,
            fill=NEG,
            base=0,
            pattern=[[-1, P]],
            channel_multiplier=1,
        )

        # ---- persistent SBUF tensors ----
        wsl = big.tile([P, EK, 3 * FPC], DT, name="wsl")
        # SWDGE on the Pool queue so it overlaps the first xT loads below
        nc.gpsimd.dma_start(wsl[:], w_dram.rearrange("(eo ei) f -> ei eo f", ei=P))
        xT = big.tile([P, EK, NT], DT, name="xT")
        qkt = big.tile([P, 2, NT], DT, name="qkt")   # [:,0,:]=Q^T  [:,1,:]=K^T
        v_nat = big.tile([P, B * KTILES, FPC], DT, name="v_nat")
        xt_view = xt_dram.rearrange("(eo ei) t -> ei eo t", ei=P)

        # ---- phase-A piece builders (each = one short PSUM-tile lifetime) --
        def pa_dma(ta, w):
            def run():
                nc.sync.dma_start(xT[:, :, ta : ta + w], xt_view[:, :, ta : ta + w])
            return run

        def pa_qk(m, ta, w):
            def run():
                qk_ps = ps.tile([P, 1024], F32, tag="sps", bufs=2, name="qk_ps")
                for e in range(EK):
                    nc.tensor.matmul(
                        qk_ps[:, 0:w],
                        lhsT=wsl[:, e, m * P : (m + 1) * P],
                        rhs=xT[:, e, ta : ta + w],
                        start=(e == 0),
                        stop=(e == EK - 1),
                    )
                nc.vector.tensor_copy(qkt[:, m, ta : ta + w], qk_ps[:, 0:w])
            return run

        def pa_v(blk, nblk):
            def run():
                v_ps = ps.tile([P, 1024], F32, tag="sps", bufs=2, name="v_ps")
                for i in range(nblk):
                    t0 = (blk + i) * P
                    for e in range(EK):
                        nc.tensor.matmul(
                            v_ps[:, i * P : (i + 1) * P],
                            lhsT=xT[:, e, t0 : t0 + P],
                            rhs=wsl[:, e, 2 * FPC : 3 * FPC],
                            start=(e == 0),
                            stop=(e == EK - 1),
                        )
                nc.vector.tensor_copy(
                    v_nat[:, blk : blk + nblk, :],
                    v_ps[:, 0 : nblk * P].rearrange("p (tb f) -> p tb f", tb=nblk),
                )
            return run

        # ---- attention helpers ----
        def s_chunk(b, k, hh, strip, coff, cw):
            """S^T matmuls (+ causal-mask PSUM seed) + exp for one <=1024-wide
            chunk of a strip."""
            q0 = P * k
            kt = qkt[hh * D : (hh + 1) * D, 1, b * T + q0 : b * T + q0 + P]
            sps = ps.tile([P, 1024], F32, tag="sps", bufs=2, name="sps")
            s = coff
            if coff == 0:
                # diagonal block: seed the mask on the PE, then accumulate S
                nc.tensor.matmul(
                    sps[:, 0:P], lhsT=tri_neg[:], rhs=id_bf[:],
                    start=True, stop=False,
                )
                nc.tensor.matmul(
                    sps[:, 0:P],
                    lhsT=kt,
                    rhs=qkt[hh * D : (hh + 1) * D, 0, b * T + q0 : b * T + q0 + P],
                    start=False,
                    stop=True,
                )
                s = P
            while s < coff + cw:
                e = min(coff + cw, (s // 512 + 1) * 512)
                nc.tensor.matmul(
                    sps[:, s - coff : e - coff],
                    lhsT=kt,
                    rhs=qkt[hh * D : (hh + 1) * D, 0, b * T + q0 + s : b * T + q0 + e],
                    start=True,
                    stop=True,
                )
                s = e
            acc = small.tile([P, 1], F32, tag="acc", name="acc")
            nc.scalar.activation(
                strip[:, coff : coff + cw],
                sps[:, :cw],
                mybir.ActivationFunctionType.Exp,
                scale=SCALE,
                accum_out=acc[:],
            )
            return acc

        def finish_head(b, k, hh, partials):
            if len(partials) == 1:
                ssum = partials[0]
            else:
                ssum = small.tile([P, 1], F32, tag="acc", name="ssum")
                nc.vector.tensor_add(ssum[:], partials[0][:], partials[1][:])
            rsum = small.tile([P, 1], F32, tag="acc", name="rsum")
            nc.vector.reciprocal(rsum[:], ssum[:])
            vp = small.tile([P, D], DT, tag="vp", name="vp")
            nc.vector.tensor_scalar_mul(
                vp[:], v_nat[:, b * KTILES + k, hh * D : (hh + 1) * D], rsum[:]
            )
            return vp

        def evac_slab(b, j, pv_ps):
            osb = outp.tile([P, 512], F32, tag="osb", name="osb")
            nc.vector.tensor_copy(osb[:], pv_ps[j][:])
            nc.sync.dma_start(
                out_dram[:, b * T + 512 * j : b * T + 512 * (j + 1)], osb[:]
            )

        def attn_batch(b, pieces_by_k, descending):
            """Attention for batch b.  pieces_by_k maps k -> list of phase-A
            piece closures emitted at the top of that iteration."""
            last_k = 0 if descending else KTILES - 1

            def pv_head(k, hh, strip, vp, pv_ps):
                q0 = P * k
                j0 = k // 4
                dead = q0 - 512 * j0
                for j in range(j0, NSLAB):
                    stop = k == (0 if descending else 4 * j + 3) and hh == HPC - 1
                    if j == j0:
                        nc.tensor.matmul(
                            pv_ps[j][hh * D : (hh + 1) * D, dead:512],
                            lhsT=vp[:],
                            rhs=strip[:, 0 : 512 - dead],
                            start=False,
                            stop=stop,
                            skip_group_check=True,
                        )
                    else:
                        nc.tensor.matmul(
                            pv_ps[j][hh * D : (hh + 1) * D, :],
                            lhsT=vp[:],
                            rhs=strip[:, 512 * j - q0 : 512 * j - q0 + 512],
                            start=False,
                            stop=stop,
                            skip_group_check=True,
                        )

            pv_ps = [
                ps.tile([P, 512], F32, tag="pv", bufs=4, name=f"pv_{b}_{j}")
                for j in range(NSLAB)
            ]
            # rhs must be initialized data (0 * NaN = NaN): wsl is loaded first
            for j in range(NSLAB):
                nc.tensor.matmul(
                    pv_ps[j][:],
                    lhsT=zeros_bf[:],
                    rhs=wsl.rearrange("p e f -> p (e f)")[:, 0:512],
                    start=True,
                    stop=False,
                    skip_group_check=True,
                )
            ks = range(KTILES - 1, -1, -1) if descending else range(KTILES)
            prev = {}
            prev_k = None
            for k in ks:
                for piece in pieces_by_k.get(k, ()):
                    piece()
                L = T - P * k
                strip_k = {}
                parts = {0: [], 1: []}
                for hh in range(HPC):
                    strip_k[hh] = strips.tile([P, T], DT, tag="strip", name=f"s{hh}")
                coff = 0
                while coff < L:
                    cw = min(1024, L - coff)
                    for hh in range(HPC):
                        parts[hh].append(s_chunk(b, k, hh, strip_k[hh], coff, cw))
                    coff += cw
                for hh in range(HPC):
                    vp = finish_head(b, k, hh, parts[hh])
                    if prev_k is not None:
                        pv_head(prev_k, hh, *prev[hh], pv_ps)
                    prev[hh] = (strip_k[hh], vp)
                if prev_k is not None and not descending and prev_k % 4 == 3:
                    evac_slab(b, prev_k // 4, pv_ps)
                prev_k = k
            for hh in range(HPC):
                pv_head(last_k, hh, *prev[hh], pv_ps)
            if descending:
                for j in range(NSLAB):
                    evac_slab(b, j, pv_ps)
            else:
                evac_slab(b, NSLAB - 1, pv_ps)

        # ---- program order ----
        # prefix: just enough of slab 3 (tokens 1920-2048) for attn0's k=15
        for piece in [
            pa_dma(1920, 128), pa_dma(1536, 384),
            pa_qk(0, 1920, 128), pa_qk(1, 1920, 128), pa_v(15, 1),
        ]:
            piece()
        # attn0 runs k DESCENDING; QKV pieces stream in by deadline.
        attn_batch(
            0,
            {
                15: [pa_qk(0, 1536, 384), pa_qk(1, 1536, 384)],
                14: [pa_v(14, 1), pa_v(12, 2)],
                13: [pa_dma(1024, 512)],
                12: [pa_qk(0, 1024, 512), pa_qk(1, 1024, 512)],
                11: [pa_v(10, 2)],
                10: [pa_v(8, 2), pa_dma(512, 512)],
                9: [pa_qk(0, 512, 512)],
                8: [pa_qk(1, 512, 512), pa_dma(3584, 512)],
                7: [pa_v(6, 2), pa_qk(0, 3584, 512)],
                6: [pa_v(4, 2), pa_qk(1, 3584, 512)],
                5: [pa_dma(0, 512), pa_dma(3072, 512)],
                4: [pa_qk(0, 0, 512), pa_qk(0, 3072, 512)],
                3: [pa_qk(1, 0, 512), pa_v(2, 2), pa_qk(1, 3072, 512), pa_dma(2560, 512)],
                2: [pa_qk(0, 2560, 512)],
                1: [pa_v(0, 2), pa_qk(1, 2560, 512), pa_dma(2048, 512)],
                0: [pa_qk(0, 2048, 512), pa_qk(1, 2048, 512)],
            },
            descending=True,
        )
        # attn1 runs k ASCENDING so its output slabs retire early (short tail)
        attn_batch(
            1,
            {
                0: [pa_v(16, 2)],
                1: [pa_v(18, 2)],
                2: [pa_v(20, 2)],
                4: [pa_v(22, 2)],
                6: [pa_v(24, 2)],
                8: [pa_v(26, 2)],
                10: [pa_v(28, 2)],
                12: [pa_v(30, 2)],
            },
            descending=False,
        )
    nc.compile()
    return nc


_NC_CACHE = None


def make_in_maps(x: np.ndarray, Ws: np.ndarray) -> list:
    xt = np.ascontiguousarray(x.reshape(NT, E).T.astype(ml_dtypes.bfloat16))
    in_maps = []
    for c in range(NCORES):
        cols = np.concatenate(
            [
                Ws[:, c * FPC : (c + 1) * FPC],
                Ws[:, E + c * FPC : E + (c + 1) * FPC],
                Ws[:, 2 * E + c * FPC : 2 * E + (c + 1) * FPC],
            ],
            axis=1,
        ).astype(ml_dtypes.bfloat16)
        in_maps.append({"xt": xt, "wsl": np.ascontiguousarray(cols)})
    return in_maps


def assemble_out(results: list) -> np.ndarray:
    out = np.empty((B, T, H * D), np.float32)
    for c in range(NCORES):
        r = results[c]["out"].reshape(FPC, B, T)
        for b in range(B):
            out[b, :, c * FPC : (c + 1) * FPC] = r[:, b, :].T
    return out


def kernel(x: np.ndarray, Ws: np.ndarray) -> np.ndarray:
    global _NC_CACHE
    if _NC_CACHE is None:
        _NC_CACHE = build_kernel()
    nc = _NC_CACHE
    res = run_bass_kernel_spmd(
        nc, make_in_maps(x, Ws), core_ids=list(range(NCORES))
    )
    return assemble_out(res.results)
